# revision 16
# baseline (speedup 1.0000x reference)
"""NVFP4-style activation quantizer on 8 TRN2 NeuronCores (raw bass).

Reference semantics (per 16-element block, fp32):
    s_t  = max|x| / (6*448)                      (global, needs all-reduce)
    m_b  = max|x| over block
    inv  = 6 / (m_b / s_t)
    s_b  = fp8_e4m3_roundtrip(inv), guarded to 1.0 if 0/inf/nan
    out  = sign(x) * fp4_121(|x|/s_t * s_b) / s_b * s_t

Device algorithm (signed, select-free), per element:
    y  = x * c            with c = s_b / s_t  (per block)
    pa = bits(y) & 0x7f800000                  (= bits of 2^e of |y|)
    B  = max_int(pa + 0x0B400000, 0x4AC00000)  (= 3*2^21 * max(2^e, 1))
    t  = y + B            (fp32 RNE add rounds y to the fp4 grid step)
    nq = B - t            (= -fp4_121(|y|)*sign(y), exact subtraction)
    out = nq * (-s_t / s_b)                    (per block)

The magic add reproduces fp4_121 rounding (incl. round-half-even ties)
because the 1-2-1 grid step is 2^(e-1) clamped to >= 0.5, which equals
ulp(3*2^21 * max(2^e,1)) in fp32.

Two passes over x in HBM: pass A computes per-block abs-max m (one
tensor_reduce per tile) and the global max (partition_all_reduce +
AllReduce-max across cores); pass B re-reads x and quantizes.
Engines: ACT issues input DMAs, SYNC issues output DMAs (both HWDGE),
DVE does all elementwise work, POOL does partition ops + the collective.
"""

import numpy as np

FULL_SHAPE = (4, 4096, 4096)
N_CORES = 8
P = 128
TOTAL = 4 * 4096 * 4096
L = TOTAL // (N_CORES * P)   # 65536 elements per partition per core
NBLK = L // 16

EXP_MASK = 0x7F800000
MAGIC_ADD = 0x0B400000       # int-bits delta for *3*2^21
MAGIC_MIN = 0x4AC00000       # bits of 6291456.0f = 3*2^21 (= B for |y|<1)


def build_nc(L=L, F=2048, n_cores=N_CORES, n_xa=3, n_o=3):
    from contextlib import ExitStack

    import concourse.bass as bass
    from concourse import mybir

    f32 = mybir.dt.float32
    i32 = mybir.dt.int32
    f8 = mybir.dt.float8e4

    T = L // F
    nblk = L // 16
    fblk = F // 16
    assert L % F == 0 and F % 16 == 0

    nc = bass.Bass(num_devices=n_cores, debug=False)
    x_ext = nc.declare_dram_parameter("x", [P, L], f32, isOutput=False)
    out_ext = nc.declare_dram_parameter("out", [P, L], f32, isOutput=True)
    cc_in = nc.dram_tensor("cc_in", [1, 128], f32)
    cc_out = nc.dram_tensor("cc_out", [1, 128], f32, addr_space="Shared")

    with ExitStack() as ctx:
        def sem(name):
            return ctx.enter_context(nc.semaphore(name))

        def sbuf(name, shape, dt=f32):
            return ctx.enter_context(nc.sbuf_tensor(name, shape, dt))

        # one sem per buffer slot: concurrent DMAs complete out of order,
        # so a shared cumulative sem cannot prove WHICH tile landed.
        s_xa = [sem(f"s_xa{i}") for i in range(n_xa)]   # in-slot DMAs  (+16)
        s_ob = [sem(f"s_ob{i}") for i in range(n_o)]    # out-slot DMAs (+16)
        s_cdma = sem("s_cdma")   # collective staging DMAs      (+16)
        s_dve = sem("s_dve")     # tagged DVE ops               (+1)
        s_pool = sem("s_pool")   # pool ops                     (+1)
        s_cc = sem("s_cc")       # collective                   (+1)
        assert T >= n_xa and T >= n_o

        xa = [sbuf(f"xa{i}", [P, F]) for i in range(n_xa)]
        yb = [sbuf(f"yb{i}", [P, F]) for i in range(2)]
        pb = [sbuf(f"pb{i}", [P, F], i32) for i in range(2)]
        tb = [sbuf(f"tb{i}", [P, F]) for i in range(2)]
        ob = [sbuf(f"ob{i}", [P, F]) for i in range(n_o)]
        m_t = sbuf("m_t", [P, nblk])
        rm_t = sbuf("rm_t", [P, nblk])
        s1_t = sbuf("s1_t", [P, nblk])
        f8_t = sbuf("f8_t", [P, nblk], f8)
        c_t = sbuf("c_t", [P, nblk])
        nic_t = sbuf("nic_t", [P, nblk])
        mx_t = sbuf("mx_t", [P, 1])
        gall_t = sbuf("gall_t", [P, 128])
        g128_t = sbuf("g128_t", [P, 1])
        st_t = sbuf("st_t", [P, 1])
        rt_t = sbuf("rt_t", [P, 1])
        nst_t = sbuf("nst_t", [P, 1])

        # DVE instruction tags (python-side bookkeeping of s_dve counts)
        dveA = [0] * T        # count after pass-A reduce of tile t
        dveB_y = [0] * T      # count after pass-B y-op of tile t (x consumed)
        dveB_o = [0] * T      # count after pass-B out-op of tile t
        K_mx_box = [0]

        with nc.Block() as block:

            @block.vector
            def _(dve):
                cnt = 0

                def inc(ins):
                    # tag the op on s_dve, then self-wait: the race detector
                    # does not extend same-engine program-order tracking
                    # across an instruction that carries a real sem update.
                    nonlocal cnt
                    ins.then_inc(s_dve)
                    cnt += 1
                    dve.wait_ge(s_dve, cnt)
                    return cnt

                # ---- pass A: per-block abs max ----
                for t in range(T):
                    dve.wait_ge(s_xa[t % n_xa], 16 * (t // n_xa + 1))
                    i = dve.tensor_reduce(
                        out=m_t[:, t * fblk:(t + 1) * fblk],
                        in_=xa[t % n_xa][:].rearrange("p (b s) -> p b s", s=16),
                        axis=mybir.AxisListType.X,
                        op=mybir.AluOpType.max,
                        apply_absolute_value=True,
                    )
                    dveA[t] = inc(i)
                # rm = 1/m (plain InstReciprocal: the only recip this
                # walrus build compiles; exactly rounded, 8 cyc/elem)
                inc(dve.reciprocal(rm_t[:], m_t[:]))
                i = dve.tensor_reduce(
                    out=mx_t[:], in_=m_t[:], axis=mybir.AxisListType.X,
                    op=mybir.AluOpType.max,
                )
                K_mx_box[0] = inc(i)

                # ---- scalars + per-block scales (after global max known) ----
                # fully serialized via s_dve (InstReciprocal and the custom
                # recip ops are unordered vs the regular stream in the race
                # detector; this phase is tiny so fencing every op is cheap)
                def step(ins):
                    inc(ins)

                dve.wait_ge(s_cdma, 32)         # gall loaded (bcast DMA)
                step(dve.tensor_reduce(
                    out=g128_t[:], in_=gall_t[:], axis=mybir.AxisListType.X,
                    op=mybir.AluOpType.max))
                step(dve.tensor_scalar(st_t[:], g128_t[:], 1.0 / 2688.0, None,
                                       op0=mybir.AluOpType.mult))
                step(dve.reciprocal(rt_t[:], st_t[:]))
                step(dve.tensor_scalar(nst_t[:], st_t[:], -1.0, None,
                                       op0=mybir.AluOpType.mult))
                # inv = rm * st * 6  (into s1)
                step(dve.tensor_scalar(s1_t[:], rm_t[:], st_t[:], 6.0,
                                       op0=mybir.AluOpType.mult,
                                       op1=mybir.AluOpType.mult))
                step(dve.tensor_copy(f8_t[:], s1_t[:]))  # fp8 round trip
                step(dve.tensor_copy(m_t[:], f8_t[:]))   # up -> m buffer
                step(dve.tensor_scalar(s1_t[:], m_t[:], 0.0, None,
                                       op0=mybir.AluOpType.is_equal))
                step(dve.tensor_tensor(rm_t[:], m_t[:], s1_t[:],
                                       op=mybir.AluOpType.add))  # s_b -> rm
                step(dve.tensor_scalar(c_t[:], rm_t[:], rt_t[:], None,
                                       op0=mybir.AluOpType.mult))
                step(dve.reciprocal(m_t[:], rm_t[:]))
                step(dve.tensor_scalar(nic_t[:], m_t[:], nst_t[:], None,
                                       op0=mybir.AluOpType.mult))

                # ---- pass B ----
                for t in range(T):
                    g = T + t
                    dve.wait_ge(s_xa[g % n_xa], 16 * (g // n_xa + 1))
                    if t >= n_o:
                        # out slot reused: wait for its DMA
                        dve.wait_ge(s_ob[t % n_o],
                                    16 * ((t - n_o) // n_o + 1))
                    y, p, tbuf, o = (yb[t % 2], pb[t % 2], tb[t % 2],
                                     ob[t % n_o])
                    bsl = slice(t * fblk, (t + 1) * fblk)
                    i = dve.tensor_tensor(
                        y[:].rearrange("p (b s) -> p b s", s=16),
                        xa[g % n_xa][:].rearrange("p (b s) -> p b s", s=16),
                        c_t[:, bsl].unsqueeze(-1).broadcast_to([P, fblk, 16]),
                        op=mybir.AluOpType.mult,
                    )
                    dveB_y[t] = inc(i)
                    inc(dve.tensor_scalar(p[:], y[:].bitcast(i32), EXP_MASK,
                                          None,
                                          op0=mybir.AluOpType.bitwise_and))
                    inc(dve.tensor_scalar(p[:], p[:], MAGIC_ADD, MAGIC_MIN,
                                          op0=mybir.AluOpType.add,
                                          op1=mybir.AluOpType.max))
                    inc(dve.tensor_tensor(tbuf[:], y[:], p[:].bitcast(f32),
                                          op=mybir.AluOpType.add))
                    inc(dve.tensor_tensor(y[:], p[:].bitcast(f32), tbuf[:],
                                          op=mybir.AluOpType.subtract))
                    i = dve.tensor_tensor(
                        o[:].rearrange("p (b s) -> p b s", s=16),
                        y[:].rearrange("p (b s) -> p b s", s=16),
                        nic_t[:, bsl].unsqueeze(-1).broadcast_to(
                            [P, fblk, 16]),
                        op=mybir.AluOpType.mult,
                    )
                    dveB_o[t] = inc(i)



            @block.scalar
            def _(act):
                # pass A input DMAs
                for t in range(T):
                    if t >= n_xa:
                        act.wait_ge(s_dve, dveA[t - n_xa])
                    act.dma_start(
                        out=xa[t % n_xa][:, :],
                        in_=x_ext[:, t * F:(t + 1) * F],
                    ).then_inc(s_xa[t % n_xa], 16)
                # pass B input DMAs (re-read)
                for t in range(T):
                    if t >= n_xa:
                        act.wait_ge(s_dve, dveB_y[t - n_xa])
                    else:
                        act.wait_ge(s_dve, dveA[T - n_xa + t])
                    act.dma_start(
                        out=xa[(T + t) % n_xa][:, :],
                        in_=x_ext[:, t * F:(t + 1) * F],
                    ).then_inc(s_xa[(T + t) % n_xa], 16)

            @block.gpsimd
            def _(pool):
                pool.wait_ge(s_cdma, 16)        # cc_in staged
                pool.collective_compute(
                    "AllReduce",
                    mybir.AluOpType.max,
                    replica_groups=[list(range(n_cores))],
                    ins=[cc_in.ap().opt()],
                    outs=[cc_out.ap().opt()],
                ).then_inc(s_cc)

            @block.sync
            def _(sync):
                sync.wait_ge(s_dve, K_mx_box[0])
                sync.dma_start(out=cc_in[:, :], in_=mx_t[:, :]).then_inc(
                    s_cdma, 16)
                sync.wait_ge(s_cc, 1)
                sync.dma_start(
                    out=gall_t[:, :],
                    in_=cc_out.ap().broadcast_to([P, 128]),
                ).then_inc(s_cdma, 16)
                for t in range(T):
                    sync.wait_ge(s_dve, dveB_o[t])
                    sync.dma_start(
                        out=out_ext[:, t * F:(t + 1) * F],
                        in_=ob[t % n_o][:, :],
                    ).then_inc(s_ob[t % n_o], 16)
                for i in range(n_o):
                    uses = len([t for t in range(T) if t % n_o == i])
                    sync.wait_ge(s_ob[i], 16 * uses)

    return nc


_CACHE = {}


def _get_nc():
    if "nc" not in _CACHE:
        _CACHE["nc"] = build_nc()
    return _CACHE["nc"]


def kernel(x: np.ndarray) -> np.ndarray:
    from concourse.bass_utils import run_bass_kernel_spmd

    x = np.asarray(x, dtype=np.float32)
    assert x.shape == FULL_SHAPE
    shards = x.reshape(N_CORES, P, L)
    in_maps = [{"x": np.ascontiguousarray(shards[i])} for i in range(N_CORES)]
    nc = _get_nc()
    res = run_bass_kernel_spmd(nc, in_maps, core_ids=list(range(N_CORES)))
    out = np.stack([r["out"] for r in res.results], axis=0)
    return out.reshape(FULL_SHAPE)
